# revision 36
# baseline (speedup 1.0000x reference)
"""Multi-head causal attention (B=2, S=2048, D=1024, H=16) on 8 TRN2 NeuronCores.

Sharding: tensor-parallel over heads x data-parallel over batch.
Core c handles batch b = c // 4 and head group g = c % 4 (heads 4g..4g+3),
i.e. a [2048, 256] slice of the output.

Per-core kernel, bf16 data path (PSUM accumulation stays fp32):
  - x, W, Q^T, K^T, V', probabilities in bf16: matmuls run 1 cycle/row at
    any width and measure faster than fp32r on hardware even with the
    explicit ldweights; DMA bytes halve; DVE copies hit the 2x 16-bit mode.
  - Q^T/K^T projections produce d-major [256, 2048] activations directly
    (lhsT = W column slice, rhs = host-pretransposed x^T); a matmul output
    may not cross a PSUM bank, so each k contributes two 512-wide matmuls
    with the weight tile stationary across both.
  - V' is k-major [128, 4*65] with a ones column per head (the PV matmul
    then also yields softmax denominators); ones columns via gpsimd memset
    on the otherwise idle Pool engine, V bias via a K=1 ones x bias-row
    f32r matmul into the same PSUM accumulation.
  - Scores are computed transposed (S^T = K @ Q^T): softmax needs no
    transposes. A head pair shares one 128-row Q^T/K^T tile; the two K=64
    score matmuls go to distinct PE row groups (tile_position (0,0)/(64,0))
    and the two halves of one [128,1024] PSUM tile, sharing a single
    strided exp on ScalarE. Causal masking is a 0/1 bf16 multiply on the
    probabilities AFTER the exp, keeping VectorE latency off the
    scores->exp chain (it only gates PV, which lags two iterations).
  - Redundant InstLdweights (consecutive matmuls sharing a stationary
    tile) are stripped post-hoc; they are unmodeled in CoreSim but cost
    real cycles on hardware.
  - Normalization: 1/s = exp(-ln(s)) on ScalarE, emitted right after a
    filler chain whose PSUM-slot hold pauses the exp stream anyway
    (custom-DVE approx reciprocal fails this walrus build, plain DVE
    reciprocal runs ~7 cycles/element on hardware, gpsimd divide is not
    a legal Pool opcode); bf16 broadcast across partitions by a K=1 ones
    matmul, one VectorE multiply.
  - Schedule: software-pipelined emission. PV lags two t-iterations behind
    its exp so upcoming scores never sit behind a stalled PV in the
    in-order PE queue; V'/projection chains are woven between t-iterations
    as PE filler, with the tail blocks carrying the NEXT loop iteration's
    lead-in chains so the steady-state body starts attention immediately.

Ruled out: fp8e4 DoubleRow PV (would halve PV stream cycles and
instruction count). Compile probes show fp8e4 exp output, f32->fp8
tensor_copy, fp8 memset and all-fp8 tensor_mul all work, but the
DoubleRow/DoubleRowSwInterleave matmul Ldweights fails walrus codegen's
ISA check in this build for every M tried (64/65/66) - the perf mode is
unsupported here, so the PV matmul cannot go below 1 cycle/row.
"""

import os
import sys

import numpy as np

for _p in ("/opt/trn_rl_repo", "/root/.axon_site/_ro/trn_rl_repo"):
    if os.path.isdir(_p) and _p not in sys.path:
        sys.path.insert(0, _p)

B, S, D, H = 2, 2048, 1024, 16
N_CORES = 8
HEADS_PER_CORE = 4
DH = D // H  # 64
DCORE = HEADS_PER_CORE * DH  # 256
KT = D // 128  # 8 contraction tiles for the projections
ST = S // 128  # 16 sequence tiles
QB = 512  # q block width
NEG = -1.0e30

_CACHE = {}


def _split_multi_waits(nc, max_waits=1):
    """This walrus build rejects instructions carrying more than one
    semaphore wait; hoist extras onto preceding NoOps on the same engine."""
    import bass_rust as _br

    n = 0
    for fn in nc.m.functions:
        for bb in fn.blocks:
            insts = list(bb.instructions)
            new = []
            changed = False
            for inst in insts:
                si = getattr(inst, "sync_info", None)
                ow = list(si.on_wait) if si is not None else []
                if len(ow) > max_waits:
                    changed = True
                    for w in ow[:-max_waits]:
                        n += 1
                        new.append(
                            _br.InstNoOp(
                                name=f"I-ws{n}",
                                engine=inst.engine,
                                ins=[],
                                outs=[],
                                sync_info=_br.SyncInfo(on_wait=[w], on_update=[]),
                            )
                        )
                    si.on_wait = ow[-max_waits:]
                    inst.sync_info = si
                new.append(inst)
            if changed:
                bb.instructions = new


def _dedup_ldweights(nc):
    """Drop an InstLdweights whose weights operand is identical to the one
    already loaded (qk_chain emits ldw/mm pairs sharing a stationary tile).
    Non-self-loading matmuls leave the PE weight registers intact; fp32/f32r
    matmuls self-load, so they invalidate the tracked key."""
    import concourse.mybir as mybir

    for fn in nc.m.functions:
        for bb in fn.blocks:
            last_key = None
            new = []
            for inst in bb.instructions:
                tn = type(inst).__name__
                if str(getattr(inst, "engine", None)) == "EngineType.PE":
                    if tn == "InstLdweights":
                        try:
                            w = inst.ins[0]
                            key = (
                                w.memref,
                                w.offset,
                                str(w.ap),
                                str(getattr(inst, "tile_position", None)),
                                str(getattr(inst, "perf_mode", None)),
                            )
                        except Exception:
                            key = None
                        si = getattr(inst, "sync_info", None)
                        waits = list(si.on_wait) if si else []
                        ups = list(si.on_update) if si else []
                        if key is not None and key == last_key and not waits and not ups:
                            continue
                        last_key = key
                    elif tn == "InstMatmult":
                        try:
                            if inst.ins[1].dtype in (
                                mybir.dt.float32,
                                mybir.dt.float32r,
                            ):
                                last_key = None
                        except Exception:
                            last_key = None
                    elif tn in ("InstNoOp", "InstEventSemaphore", "InstRegisterMove"):
                        pass
                    else:
                        last_key = None
                new.append(inst)
            bb.instructions = new


def build_module(repeat=1, hw_loop=False):
    import contextlib

    import concourse.bass as bass
    import concourse.mybir as mybir
    from concourse.tile import TileContext

    F32 = mybir.dt.float32
    F32R = mybir.dt.float32r
    BF16 = mybir.dt.bfloat16
    AF = mybir.ActivationFunctionType

    nc = bass.Bass("TRN2", target_bir_lowering=False, debug=False, num_devices=N_CORES)

    xT_in = nc.declare_dram_parameter("xT", [D, S], BF16, isOutput=False)
    wq_in = nc.declare_dram_parameter("wq", [D, DCORE], BF16, isOutput=False)
    wk_in = nc.declare_dram_parameter("wk", [D, DCORE], BF16, isOutput=False)
    wv_in = nc.declare_dram_parameter("wv", [D, DCORE], BF16, isOutput=False)
    bq_in = nc.declare_dram_parameter("bq", [DCORE], F32, isOutput=False)
    bk_in = nc.declare_dram_parameter("bk", [DCORE], F32, isOutput=False)
    bv_in = nc.declare_dram_parameter("bv", [DCORE], F32, isOutput=False)
    tri_in = nc.declare_dram_parameter("tri", [128, 256], BF16, isOutput=False)
    ones_in = nc.declare_dram_parameter("ones", [128, 4], F32, isOutput=False)
    outT = nc.declare_dram_parameter("outT", [DCORE, S], F32, isOutput=True)

    with TileContext(nc) as tc:
        with (
            tc.tile_pool(name="persist", bufs=1) as pp,
            tc.tile_pool(name="work", bufs=8) as wp,
            tc.tile_pool(name="outp", bufs=6) as op,
            tc.tile_pool(name="mm_ps", bufs=2, space="PSUM") as mm_ps,
            tc.tile_pool(name="att_ps", bufs=2, space="PSUM") as att_ps,
        ):
            # ---- constant / persistent tiles -------------------------------
            trid = pp.tile([128, 256], BF16, tag="trid")
            nc.sync.dma_start(trid[:], tri_in[:])
            onesr = pp.tile([1, 128], F32R, tag="onesr")  # K=1 matmul lhsT
            nc.sync.dma_start(
                onesr[:], ones_in[:, 0:1].rearrange("p a -> a p").bitcast(F32R)
            )
            bvrow = pp.tile([1, DCORE], F32R, tag="bvrow")
            nc.sync.dma_start(
                bvrow[:], bv_in[:].rearrange("(a b) -> a b", a=1).bitcast(F32R)
            )
            bqc = pp.tile([128, 2], F32, tag="bqc")
            nc.sync.dma_start(bqc[:], bq_in[:].rearrange("(m p) -> p m", p=128))
            bkc = pp.tile([128, 2], F32, tag="bkc")
            nc.sync.dma_start(bkc[:], bk_in[:].rearrange("(m p) -> p m", p=128))
            onesb = pp.tile([1, 64], BF16, tag="onesb")  # bf16 bcast lhsT
            nc.vector.tensor_copy(onesb[:], onesr[:, 0:64].bitcast(F32))

            wq = []
            wk = []
            wv = []
            for k in range(KT):
                for name, lst, src in (("wq", wq, wq_in), ("wk", wk, wk_in), ("wv", wv, wv_in)):
                    t = pp.tile([128, DCORE], BF16, tag=f"{name}{k}")
                    nc.sync.dma_start(t[:], src[128 * k : 128 * (k + 1), :])
                    lst.append(t)
            # ---- warmup during the x DMA window: dummy matmuls ramp the PE
            # HAM clock gate to 2.4 GHz; one exp pulls the activation table
            # load off the critical path ------------------------------------
            warm_ps = mm_ps.tile([128, 2 * QB], F32, tag="mm", name="warm_ps")
            for _w in range(42):
                nc.tensor.matmul(
                    warm_ps[:, 0:DCORE], onesr[:], bvrow[:], start=True, stop=True
                )
            warm_o = wp.tile([1, 128], F32, tag="warm", name="warm_o")
            nc.scalar.activation(warm_o[:], onesr[:].bitcast(F32), AF.Exp)

            # x^T tiles, loaded in [128, QB] slices n-major so the first
            # projection blocks can start after ~1/4 of x has landed
            xt = [pp.tile([128, S], BF16, tag=f"xt{k}", name=f"xt{k}") for k in range(KT)]
            for n in range(S // QB):
                for k in range(KT):
                    nc.sync.dma_start(
                        xt[k][:, QB * n : QB * (n + 1)],
                        xT_in[128 * k : 128 * (k + 1), QB * n : QB * (n + 1)],
                    )

            qT = [pp.tile([128, S], BF16, tag=f"qT{m}", name=f"qT{m}") for m in range(2)]
            kTt = [pp.tile([128, S], BF16, tag=f"kT{m}", name=f"kT{m}") for m in range(2)]
            vp = [pp.tile([128, 4 * 65], BF16, tag=f"vp{s}", name=f"vp{s}") for s in range(ST)]

            def v_chain(s):
                # one V' sequence tile: k-major [128, 4*65] with a ones
                # column per head (PV then also yields softmax denoms)
                dst = vp[s]
                nc.gpsimd.memset(
                    dst[:].rearrange("p (h c) -> p h c", c=65)[:, :, 64:65], 1.0
                )
                ps = mm_ps.tile([128, DCORE], F32, tag="mm", name=f"vps{s}")
                for k in range(KT):
                    nc.tensor.matmul(
                        ps[:],
                        xt[k][:, 128 * s : 128 * (s + 1)],
                        wv[k][:],
                        start=(k == 0),
                        stop=False,
                    )
                nc.tensor.matmul(ps[:], onesr[:], bvrow[:], start=False, stop=True)
                nc.vector.tensor_copy(
                    dst[:].rearrange("p (h c) -> p h c", c=65)[:, :, 0:64],
                    ps[:].rearrange("p (h c) -> p h c", c=64),
                )

            def qk_chain(lst, w, bias, m, half):
                # one [128,1024]-wide accumulation chain of a Q/K proj;
                # matmul outputs may not cross a PSUM bank, so each k
                # contributes two 512-wide matmuls (weights stationary)
                acc = mm_ps.tile([128, 2 * QB], F32, tag="mm", name="acc")
                for k in range(KT):
                    for h2 in range(2):
                        nc.tensor.matmul(
                            acc[:, QB * h2 : QB * (h2 + 1)],
                            w[k][:, 128 * m : 128 * (m + 1)],
                            xt[k][
                                :,
                                2 * QB * half + QB * h2 : 2 * QB * half
                                + QB * (h2 + 1),
                            ],
                            start=(k == 0),
                            stop=(k == KT - 1),
                        )
                nc.vector.tensor_scalar_add(
                    lst[m][:, 2 * QB * half : 2 * QB * (half + 1)],
                    acc[:],
                    bias[:, m : m + 1],
                )

            # ---- lead-in (once): V' s0-3 + Q/K m0 half0 unlock the first two
            # attention blocks; in the repeat loop these chains for the NEXT
            # iteration are emitted as fillers in the tail blocks, so the
            # steady-state body starts its attention immediately ------------
            v_chain(0)
            v_chain(1)
            v_chain(2)
            v_chain(3)
            qk_chain(qT, wq, bqc, 0, 0)
            qk_chain(kTt, wk, bkc, 0, 0)

            if hw_loop and repeat > 1:
                rep_iter = [0]
                rep_ctx = tc.For_i(0, repeat, 1)
            else:
                rep_iter = range(repeat)
                rep_ctx = contextlib.nullcontext()
            with rep_ctx:
              for _rep in rep_iter:
                # filler chains woven between t-iterations (one per two t's),
                # ordered so each completes before its consumer block; the
                # tail blocks carry the next iteration's lead-in chains
                per_block_fillers = {
                    0: [
                        (False, lambda: v_chain(4)),
                        (True, lambda: qk_chain(qT, wq, bqc, 1, 0)),
                    ],
                    1: [
                        (False, lambda: v_chain(5)),
                        (True, lambda: qk_chain(kTt, wk, bkc, 1, 0)),
                        (False, lambda: v_chain(6)),
                        (False, lambda: v_chain(7)),
                    ],
                    2: [
                        (True, lambda: qk_chain(qT, wq, bqc, 0, 1)),
                        (True, lambda: qk_chain(kTt, wk, bkc, 0, 1)),
                    ],
                    3: [
                        (False, lambda: v_chain(8)),
                        (False, lambda: v_chain(9)),
                        (False, lambda: v_chain(10)),
                        (False, lambda: v_chain(11)),
                    ],
                    4: [
                        (True, lambda: qk_chain(qT, wq, bqc, 1, 1)),
                        (True, lambda: qk_chain(kTt, wk, bkc, 1, 1)),
                        (False, lambda: v_chain(12)),
                        (False, lambda: v_chain(13)),
                        (False, lambda: v_chain(14)),
                        (False, lambda: v_chain(15)),
                    ],
                    6: [
                        (True, lambda: qk_chain(qT, wq, bqc, 0, 0)),
                        (True, lambda: qk_chain(kTt, wk, bkc, 0, 0)),
                    ],
                    7: [
                        (False, lambda: v_chain(0)),
                        (False, lambda: v_chain(1)),
                        (True, lambda: v_chain(2)),
                        (False, lambda: v_chain(3)),
                    ],
                }

                # ---- attention, software-pipelined emission ---------------
                blocks = [(0, 0), (0, 1), (1, 0), (1, 1), (0, 2), (0, 3), (1, 2), (1, 3)]
                prev_pv = None  # (aps, hA, hB, pt, qoff, t, last)
                pending_norms = []  # FIFO of (aps, hA, hB, j)

                def emit_pv():
                    nonlocal prev_pv
                    if prev_pv is None:
                        return
                    aps, hA, hB, pt, qoff, t, last = prev_pv
                    for h, off in ((hA, 0), (hB, QB)):
                        nc.tensor.matmul(
                            aps[0:65, off + qoff : off + QB],
                            vp[t][:, 65 * h : 65 * h + 65],
                            pt[:, off + qoff : off + QB],
                            start=(t == 0),
                            stop=last,
                        )
                    prev_pv = None

                def emit_norm():
                    if not pending_norms:
                        return
                    aps, hA, hB, j = pending_norms.pop(0)
                    # 1/s = exp(-ln(s)) on ScalarE, emitted right after a
                    # filler chain whose PSUM-slot hold pauses the exp
                    # stream anyway. (Alternatives that fail here: custom-DVE
                    # approx reciprocal -> "ISA wrong length" in this walrus
                    # build; plain DVE reciprocal -> ~7 cycles/element on
                    # hardware; gpsimd divide -> not a legal Pool opcode.)
                    lns = wp.tile([1, 2 * QB], F32, tag="lns", name="lns")
                    nc.scalar.activation(lns[:], aps[64:65, :], AF.Ln)
                    rrow = wp.tile([1, 2 * QB], BF16, tag="rrow", name="rrow")
                    nc.scalar.activation(rrow[:], lns[:], AF.Exp, scale=-1.0)
                    rbp = mm_ps.tile([128, 2 * QB], F32, tag="mm", name="rbp")
                    for h2 in range(2):
                        nc.tensor.matmul(
                            rbp[0:64, QB * h2 : QB * (h2 + 1)],
                            onesb[:],
                            rrow[:, QB * h2 : QB * (h2 + 1)],
                            start=True,
                            stop=True,
                        )
                    rb = wp.tile([64, 2 * QB], F32, tag="rb", name="rb")
                    nc.vector.tensor_copy(rb[:], rbp[0:64, :])
                    att = op.tile([64, 2 * QB], F32, tag="att_out", name="att")
                    nc.vector.tensor_mul(att[:], aps[0:64, :], rb[:])
                    for h, off in ((hA, 0), (hB, QB)):
                        nc.sync.dma_start(
                            outT[64 * h : 64 * (h + 1), QB * j : QB * (j + 1)],
                            att[:, off : off + QB],
                        )

                for bi, (hp, j) in enumerate(blocks):
                    hA, hB = 2 * hp, 2 * hp + 1
                    qTm, kTm = qT[hp], kTt[hp]
                    bfill = list(per_block_fillers.get(bi, ()))
                    aps = att_ps.tile([128, 2 * QB], F32, tag="att", name=f"aps{hp}_{j}")
                    for t in range(4 * j + 4):
                        i = t - 4 * j  # >= 0 only on diagonal-region tiles
                        qoff = 128 * max(i, 0)
                        qwin = slice(QB * j + qoff, QB * (j + 1))
                        ktile = slice(128 * t, 128 * (t + 1))
                        sps = mm_ps.tile([128, 2 * QB], F32, tag="mm", name="sps")
                        nc.tensor.matmul(
                            sps[:, qoff:QB],
                            kTm[0:64, ktile],
                            qTm[0:64, qwin],
                            start=True,
                            stop=True,
                            tile_position=(0, 0),
                        )
                        nc.tensor.matmul(
                            sps[:, QB + qoff : 2 * QB],
                            kTm[64:128, ktile],
                            qTm[64:128, qwin],
                            start=True,
                            stop=True,
                            tile_position=(64, 0),
                        )
                        spsv = sps[:].rearrange("p (two c) -> p two c", two=2)
                        pt = wp.tile([128, 2 * QB], BF16, tag="pt")
                        ptv = pt[:].rearrange("p (two c) -> p two c", two=2)
                        nc.scalar.activation(
                            ptv[:, :, qoff:QB],
                            spsv[:, :, qoff:QB],
                            AF.Exp,
                            scale=float(1.0 / np.sqrt(DH)),
                        )
                        if i >= 0:
                            # causal mask applied post-exp as a 0/1 multiply:
                            # keeps the DVE op off the scores->exp chain (it
                            # only gates PV, which lags two iterations)
                            nc.vector.tensor_mul(
                                ptv[:, :, qoff : qoff + 128],
                                ptv[:, :, qoff : qoff + 128],
                                trid[:].rearrange("p (two c) -> p two c", two=2),
                            )
                        emit_pv()
                        prev_pv = (aps, hA, hB, pt, qoff, t, t == 4 * j + 3)
                        if t % 2 == 1 and bfill:
                            bfill.pop(0)[1]()
                            emit_norm()
                        elif t == 3:
                            emit_norm()
                    for _, f in bfill:
                        f()
                    emit_pv()
                    pending_norms.append((aps, hA, hB, j))
                while pending_norms:
                    emit_norm()

    _dedup_ldweights(nc)
    _split_multi_waits(nc)
    return nc


def _get_runner():
    if "nc" not in _CACHE:
        _CACHE["nc"] = build_module()
    return _CACHE["nc"]


def _make_in_maps(x, Wq, bq, Wk, bk, Wv, bv):
    import ml_dtypes

    bf16 = ml_dtypes.bfloat16
    x = np.asarray(x, dtype=np.float32)
    Wq = np.asarray(Wq, dtype=bf16)
    Wk = np.asarray(Wk, dtype=bf16)
    Wv = np.asarray(Wv, dtype=bf16)
    bq = np.asarray(bq, dtype=np.float32)
    bk = np.asarray(bk, dtype=np.float32)
    bv = np.asarray(bv, dtype=np.float32)

    kp = np.arange(128)[:, None]
    qf = np.arange(128)[None, :]
    tri = np.where(kp <= qf, 1.0, 0.0).astype(bf16)
    trid = np.concatenate([tri, tri], axis=1)
    ones = np.ones((128, 4), np.float32)

    xTs = [np.ascontiguousarray(x[b].T.astype(bf16)) for b in range(B)]
    in_maps = []
    for c in range(N_CORES):
        b = c // 4
        g = c % 4
        sl = slice(DCORE * g, DCORE * (g + 1))
        in_maps.append(
            {
                "xT": xTs[b],
                "wq": np.ascontiguousarray(Wq[:, sl]),
                "wk": np.ascontiguousarray(Wk[:, sl]),
                "wv": np.ascontiguousarray(Wv[:, sl]),
                "bq": np.ascontiguousarray(bq[sl]),
                "bk": np.ascontiguousarray(bk[sl]),
                "bv": np.ascontiguousarray(bv[sl]),
                "tri": trid,
                "ones": ones,
            }
        )
    return in_maps


def kernel(x, Wq, bq, Wk, bk, Wv, bv):
    from concourse.bass_utils import run_bass_kernel_spmd

    nc = _get_runner()
    in_maps = _make_in_maps(x, Wq, bq, Wk, bk, Wv, bv)
    res = run_bass_kernel_spmd(nc, in_maps, list(range(N_CORES)))
    out = np.empty((B, S, D), dtype=np.float32)
    for c in range(N_CORES):
        b = c // 4
        g = c % 4
        out[b, :, DCORE * g : DCORE * (g + 1)] = res.results[c]["outT"].T
    return out


# revision 40
# speedup vs baseline: 1.0387x; 1.0387x over previous
"""Multi-head causal attention (B=2, S=2048, D=1024, H=16) on 8 TRN2 NeuronCores.

Sharding: tensor-parallel over heads x data-parallel over batch.
Core c handles batch b = c // 4 and head group g = c % 4 (heads 4g..4g+3),
i.e. a [2048, 256] slice of the output.

Per-core kernel, bf16 data path (PSUM accumulation stays fp32):
  - x, W, Q^T, K^T, V', probabilities in bf16: matmuls run 1 cycle/row at
    any width and measure faster than fp32r on hardware even with the
    explicit ldweights; DMA bytes halve; DVE copies hit the 2x 16-bit mode.
  - Q^T/K^T projections produce d-major [256, 2048] activations directly
    (lhsT = W column slice, rhs = host-pretransposed x^T); a matmul output
    may not cross a PSUM bank, so each k contributes two 512-wide matmuls
    with the weight tile stationary across both.
  - V' is k-major [128, 4*65] with a ones column per head (the PV matmul
    then also yields softmax denominators); ones columns via gpsimd memset
    on the otherwise idle Pool engine, V bias via a K=1 ones x bias-row
    f32r matmul into the same PSUM accumulation.
  - Scores are computed transposed (S^T = K @ Q^T): softmax needs no
    transposes. A head pair shares one 128-row Q^T/K^T tile; the two K=64
    score matmuls go to distinct PE row groups (tile_position (0,0)/(64,0))
    and the two halves of one [128,1024] PSUM tile, sharing a single
    strided exp on ScalarE. Causal masking is a 0/1 bf16 multiply on the
    probabilities AFTER the exp, keeping VectorE latency off the
    scores->exp chain (it only gates PV, which lags two iterations).
  - Redundant InstLdweights (consecutive matmuls sharing a stationary
    tile) are stripped post-hoc; they are unmodeled in CoreSim but cost
    real cycles on hardware.
  - Normalization: 1/s = exp(-ln(s)) on ScalarE, emitted right after a
    filler chain whose PSUM-slot hold pauses the exp stream anyway
    (custom-DVE approx reciprocal fails this walrus build, plain DVE
    reciprocal runs ~7 cycles/element on hardware, gpsimd divide is not
    a legal Pool opcode); bf16 broadcast across partitions by a K=1 ones
    matmul, one VectorE multiply.
  - Schedule: software-pipelined emission. PV lags two t-iterations behind
    its exp so upcoming scores never sit behind a stalled PV in the
    in-order PE queue; V'/projection chains are woven between t-iterations
    as PE filler, with the tail blocks carrying the NEXT loop iteration's
    lead-in chains so the steady-state body starts attention immediately.

Ruled out: fp8e4 DoubleRow PV (would halve PV stream cycles and
instruction count). Compile probes show fp8e4 exp output, f32->fp8
tensor_copy, fp8 memset and all-fp8 tensor_mul all work, but the
DoubleRow/DoubleRowSwInterleave matmul Ldweights fails walrus codegen's
ISA check in this build for every M tried (64/65/66) - the perf mode is
unsupported here, so the PV matmul cannot go below 1 cycle/row.
"""

import os
import sys

import numpy as np

for _p in ("/opt/trn_rl_repo", "/root/.axon_site/_ro/trn_rl_repo"):
    if os.path.isdir(_p) and _p not in sys.path:
        sys.path.insert(0, _p)

B, S, D, H = 2, 2048, 1024, 16
N_CORES = 8
HEADS_PER_CORE = 4
DH = D // H  # 64
DCORE = HEADS_PER_CORE * DH  # 256
KT = D // 128  # 8 contraction tiles for the projections
ST = S // 128  # 16 sequence tiles
QB = 512  # q block width
NEG = -1.0e30

_CACHE = {}


def _split_multi_waits(nc, max_waits=1):
    """This walrus build rejects instructions carrying more than one
    semaphore wait; hoist extras onto preceding NoOps on the same engine."""
    import bass_rust as _br

    n = 0
    for fn in nc.m.functions:
        for bb in fn.blocks:
            insts = list(bb.instructions)
            new = []
            changed = False
            for inst in insts:
                si = getattr(inst, "sync_info", None)
                ow = list(si.on_wait) if si is not None else []
                if len(ow) > max_waits:
                    changed = True
                    for w in ow[:-max_waits]:
                        n += 1
                        new.append(
                            _br.InstNoOp(
                                name=f"I-ws{n}",
                                engine=inst.engine,
                                ins=[],
                                outs=[],
                                sync_info=_br.SyncInfo(on_wait=[w], on_update=[]),
                            )
                        )
                    si.on_wait = ow[-max_waits:]
                    inst.sync_info = si
                new.append(inst)
            if changed:
                bb.instructions = new


def _dedup_ldweights(nc):
    """Drop an InstLdweights whose weights operand is identical to the one
    already loaded (qk_chain emits ldw/mm pairs sharing a stationary tile).
    Non-self-loading matmuls leave the PE weight registers intact; fp32/f32r
    matmuls self-load, so they invalidate the tracked key."""
    import concourse.mybir as mybir

    for fn in nc.m.functions:
        for bb in fn.blocks:
            last_key = None
            new = []
            for inst in bb.instructions:
                tn = type(inst).__name__
                if str(getattr(inst, "engine", None)) == "EngineType.PE":
                    if tn == "InstLdweights":
                        try:
                            w = inst.ins[0]
                            key = (
                                w.memref,
                                w.offset,
                                str(w.ap),
                                str(getattr(inst, "tile_position", None)),
                                str(getattr(inst, "perf_mode", None)),
                            )
                        except Exception:
                            key = None
                        si = getattr(inst, "sync_info", None)
                        waits = list(si.on_wait) if si else []
                        ups = list(si.on_update) if si else []
                        if key is not None and key == last_key and not waits and not ups:
                            continue
                        last_key = key
                    elif tn == "InstMatmult":
                        try:
                            if inst.ins[1].dtype in (
                                mybir.dt.float32,
                                mybir.dt.float32r,
                            ):
                                last_key = None
                        except Exception:
                            last_key = None
                    elif tn in ("InstNoOp", "InstEventSemaphore", "InstRegisterMove"):
                        pass
                    else:
                        last_key = None
                new.append(inst)
            bb.instructions = new


def build_module(repeat=1, hw_loop=False):
    import contextlib

    import concourse.bass as bass
    import concourse.mybir as mybir
    from concourse.tile import TileContext

    F32 = mybir.dt.float32
    F32R = mybir.dt.float32r
    BF16 = mybir.dt.bfloat16
    AF = mybir.ActivationFunctionType

    nc = bass.Bass("TRN2", target_bir_lowering=False, debug=False, num_devices=N_CORES)

    xT_in = nc.declare_dram_parameter("xT", [D, S], BF16, isOutput=False)
    wq_in = nc.declare_dram_parameter("wq", [D, DCORE], BF16, isOutput=False)
    wk_in = nc.declare_dram_parameter("wk", [D, DCORE], BF16, isOutput=False)
    wv_in = nc.declare_dram_parameter("wv", [D, DCORE], BF16, isOutput=False)
    bq_in = nc.declare_dram_parameter("bq", [DCORE], F32, isOutput=False)
    bk_in = nc.declare_dram_parameter("bk", [DCORE], F32, isOutput=False)
    bv_in = nc.declare_dram_parameter("bv", [DCORE], F32, isOutput=False)
    tri_in = nc.declare_dram_parameter("tri", [128, 256], BF16, isOutput=False)
    ones_in = nc.declare_dram_parameter("ones", [128, 4], F32, isOutput=False)
    outT = nc.declare_dram_parameter("outT", [DCORE, S], F32, isOutput=True)

    with TileContext(nc) as tc:
        with (
            tc.tile_pool(name="persist", bufs=1) as pp,
            tc.tile_pool(name="work", bufs=4) as wp,
            tc.tile_pool(name="outp", bufs=3) as op,
            tc.tile_pool(name="mm_ps", bufs=2, space="PSUM") as mm_ps,
            tc.tile_pool(name="att_ps", bufs=2, space="PSUM") as att_ps,
        ):
            # ---- constant / persistent tiles -------------------------------
            trid = pp.tile([128, 256], BF16, tag="trid")
            nc.sync.dma_start(trid[:], tri_in[:])
            onesr = pp.tile([1, 128], F32R, tag="onesr")  # K=1 matmul lhsT
            nc.sync.dma_start(
                onesr[:], ones_in[:, 0:1].rearrange("p a -> a p").bitcast(F32R)
            )
            bvrow = pp.tile([1, DCORE], F32R, tag="bvrow")
            nc.sync.dma_start(
                bvrow[:], bv_in[:].rearrange("(a b) -> a b", a=1).bitcast(F32R)
            )
            bqc = pp.tile([128, 2], F32, tag="bqc")
            nc.sync.dma_start(bqc[:], bq_in[:].rearrange("(m p) -> p m", p=128))
            bkc = pp.tile([128, 2], F32, tag="bkc")
            nc.sync.dma_start(bkc[:], bk_in[:].rearrange("(m p) -> p m", p=128))
            onesb = pp.tile([1, 64], BF16, tag="onesb")  # bf16 bcast lhsT
            nc.vector.tensor_copy(onesb[:], onesr[:, 0:64].bitcast(F32))

            wq = []
            wk = []
            wv = []
            for k in range(KT):
                for name, lst, src in (("wq", wq, wq_in), ("wk", wk, wk_in), ("wv", wv, wv_in)):
                    t = pp.tile([128, DCORE], BF16, tag=f"{name}{k}")
                    nc.sync.dma_start(t[:], src[128 * k : 128 * (k + 1), :])
                    lst.append(t)
            # ---- warmup during the x DMA window: dummy matmuls ramp the PE
            # HAM clock gate to 2.4 GHz; one exp pulls the activation table
            # load off the critical path ------------------------------------
            warm_ps = mm_ps.tile([128, 2 * QB], F32, tag="mm", name="warm_ps")
            for _w in range(42):
                nc.tensor.matmul(
                    warm_ps[:, 0:DCORE], onesr[:], bvrow[:], start=True, stop=True
                )
            warm_o = wp.tile([1, 128], F32, tag="warm", name="warm_o")
            nc.scalar.activation(warm_o[:], onesr[:].bitcast(F32), AF.Exp)

            # x^T tiles, loaded in [128, QB] slices n-major so the first
            # projection blocks can start after ~1/4 of x has landed
            xt = [pp.tile([128, S], BF16, tag=f"xt{k}", name=f"xt{k}") for k in range(KT)]
            for n in range(S // QB):
                for k in range(KT):
                    nc.sync.dma_start(
                        xt[k][:, QB * n : QB * (n + 1)],
                        xT_in[128 * k : 128 * (k + 1), QB * n : QB * (n + 1)],
                    )

            qT = [pp.tile([128, S], BF16, tag=f"qT{m}", name=f"qT{m}") for m in range(2)]
            kTt = [pp.tile([128, S], BF16, tag=f"kT{m}", name=f"kT{m}") for m in range(2)]
            vp = [pp.tile([128, 4 * 65], BF16, tag=f"vp{s}", name=f"vp{s}") for s in range(ST)]

            def v_chain(s):
                # one V' sequence tile: k-major [128, 4*65] with a ones
                # column per head (PV then also yields softmax denoms)
                dst = vp[s]
                nc.gpsimd.memset(
                    dst[:].rearrange("p (h c) -> p h c", c=65)[:, :, 64:65], 1.0
                )
                ps = mm_ps.tile([128, DCORE], F32, tag="mm", name=f"vps{s}")
                for k in range(KT):
                    nc.tensor.matmul(
                        ps[:],
                        xt[k][:, 128 * s : 128 * (s + 1)],
                        wv[k][:],
                        start=(k == 0),
                        stop=False,
                    )
                nc.tensor.matmul(ps[:], onesr[:], bvrow[:], start=False, stop=True)
                nc.vector.tensor_copy(
                    dst[:].rearrange("p (h c) -> p h c", c=65)[:, :, 0:64],
                    ps[:].rearrange("p (h c) -> p h c", c=64),
                )

            def qk_chain(lst, w, bias, m, half):
                # one [128,1024]-wide accumulation chain of a Q/K proj;
                # matmul outputs may not cross a PSUM bank, so each k
                # contributes two 512-wide matmuls (weights stationary)
                acc = mm_ps.tile([128, 2 * QB], F32, tag="mm", name="acc")
                for k in range(KT):
                    for h2 in range(2):
                        nc.tensor.matmul(
                            acc[:, QB * h2 : QB * (h2 + 1)],
                            w[k][:, 128 * m : 128 * (m + 1)],
                            xt[k][
                                :,
                                2 * QB * half + QB * h2 : 2 * QB * half
                                + QB * (h2 + 1),
                            ],
                            start=(k == 0),
                            stop=(k == KT - 1),
                        )
                nc.vector.tensor_scalar_add(
                    lst[m][:, 2 * QB * half : 2 * QB * (half + 1)],
                    acc[:],
                    bias[:, m : m + 1],
                )

            # ---- lead-in (once): V' s0-3 + Q/K m0 half0 unlock the first two
            # attention blocks; in the repeat loop these chains for the NEXT
            # iteration are emitted as fillers in the tail blocks, so the
            # steady-state body starts its attention immediately ------------
            v_chain(0)
            v_chain(1)
            v_chain(2)
            v_chain(3)
            qk_chain(qT, wq, bqc, 0, 0)
            qk_chain(kTt, wk, bkc, 0, 0)

            def emit_body():
                # filler chains woven between t-iterations (one per two t's),
                # ordered so each completes before its consumer block; the
                # tail blocks carry the next iteration's lead-in chains
                per_block_fillers = {
                    0: [
                        (False, lambda: v_chain(4)),
                        (True, lambda: qk_chain(qT, wq, bqc, 1, 0)),
                    ],
                    1: [
                        (False, lambda: v_chain(5)),
                        (True, lambda: qk_chain(kTt, wk, bkc, 1, 0)),
                        (False, lambda: v_chain(6)),
                        (False, lambda: v_chain(7)),
                    ],
                    2: [
                        (True, lambda: qk_chain(qT, wq, bqc, 0, 1)),
                        (True, lambda: qk_chain(kTt, wk, bkc, 0, 1)),
                    ],
                    3: [
                        (False, lambda: v_chain(8)),
                        (False, lambda: v_chain(9)),
                        (False, lambda: v_chain(10)),
                        (False, lambda: v_chain(11)),
                    ],
                    4: [
                        (True, lambda: qk_chain(qT, wq, bqc, 1, 1)),
                        (True, lambda: qk_chain(kTt, wk, bkc, 1, 1)),
                        (False, lambda: v_chain(12)),
                        (False, lambda: v_chain(13)),
                        (False, lambda: v_chain(14)),
                        (False, lambda: v_chain(15)),
                    ],
                    6: [
                        (True, lambda: qk_chain(qT, wq, bqc, 0, 0)),
                        (True, lambda: qk_chain(kTt, wk, bkc, 0, 0)),
                    ],
                    7: [
                        (False, lambda: v_chain(0)),
                        (False, lambda: v_chain(1)),
                        (True, lambda: v_chain(2)),
                        (False, lambda: v_chain(3)),
                    ],
                }

                # ---- attention, software-pipelined emission ---------------
                blocks = [(0, 0), (0, 1), (1, 0), (1, 1), (0, 2), (0, 3), (1, 2), (1, 3)]
                prev_pv = None  # (aps, hA, hB, pt, qoff, t, last)
                pending_norms = []  # FIFO of (aps, hA, hB, j)

                def emit_pv():
                    nonlocal prev_pv
                    if prev_pv is None:
                        return
                    aps, hA, hB, pt, qoff, t, last = prev_pv
                    for h, off in ((hA, 0), (hB, QB)):
                        nc.tensor.matmul(
                            aps[0:65, off + qoff : off + QB],
                            vp[t][:, 65 * h : 65 * h + 65],
                            pt[:, off + qoff : off + QB],
                            start=(t == 0),
                            stop=last,
                        )
                    prev_pv = None

                def emit_norm():
                    if not pending_norms:
                        return
                    aps, hA, hB, j = pending_norms.pop(0)
                    # 1/s = exp(-ln(s)) on ScalarE, emitted right after a
                    # filler chain whose PSUM-slot hold pauses the exp
                    # stream anyway. (Alternatives that fail here: custom-DVE
                    # approx reciprocal -> "ISA wrong length" in this walrus
                    # build; plain DVE reciprocal -> ~7 cycles/element on
                    # hardware; gpsimd divide -> not a legal Pool opcode.)
                    lns = wp.tile([1, 2 * QB], F32, tag="lns", name="lns")
                    nc.scalar.activation(lns[:], aps[64:65, :], AF.Ln)
                    rrow = wp.tile([1, 2 * QB], BF16, tag="rrow", name="rrow")
                    nc.scalar.activation(rrow[:], lns[:], AF.Exp, scale=-1.0)
                    rbp = mm_ps.tile([128, 2 * QB], F32, tag="mm", name="rbp")
                    for h2 in range(2):
                        nc.tensor.matmul(
                            rbp[0:64, QB * h2 : QB * (h2 + 1)],
                            onesb[:],
                            rrow[:, QB * h2 : QB * (h2 + 1)],
                            start=True,
                            stop=True,
                        )
                    rb = wp.tile([64, 2 * QB], F32, tag="rb", name="rb")
                    nc.vector.tensor_copy(rb[:], rbp[0:64, :])
                    att = op.tile([64, 2 * QB], F32, tag="att_out", name="att")
                    nc.vector.tensor_mul(att[:], aps[0:64, :], rb[:])
                    for h, off in ((hA, 0), (hB, QB)):
                        nc.sync.dma_start(
                            outT[64 * h : 64 * (h + 1), QB * j : QB * (j + 1)],
                            att[:, off : off + QB],
                        )

                for bi, (hp, j) in enumerate(blocks):
                    hA, hB = 2 * hp, 2 * hp + 1
                    qTm, kTm = qT[hp], kTt[hp]
                    bfill = list(per_block_fillers.get(bi, ()))
                    aps = att_ps.tile([128, 2 * QB], F32, tag="att", name=f"aps{hp}_{j}")
                    for t in range(4 * j + 4):
                        i = t - 4 * j  # >= 0 only on diagonal-region tiles
                        qoff = 128 * max(i, 0)
                        qwin = slice(QB * j + qoff, QB * (j + 1))
                        ktile = slice(128 * t, 128 * (t + 1))
                        sps = mm_ps.tile([128, 2 * QB], F32, tag="mm", name="sps")
                        nc.tensor.matmul(
                            sps[:, qoff:QB],
                            kTm[0:64, ktile],
                            qTm[0:64, qwin],
                            start=True,
                            stop=True,
                            tile_position=(0, 0),
                        )
                        nc.tensor.matmul(
                            sps[:, QB + qoff : 2 * QB],
                            kTm[64:128, ktile],
                            qTm[64:128, qwin],
                            start=True,
                            stop=True,
                            tile_position=(64, 0),
                        )
                        spsv = sps[:].rearrange("p (two c) -> p two c", two=2)
                        pt = wp.tile([128, 2 * QB], BF16, tag="pt")
                        ptv = pt[:].rearrange("p (two c) -> p two c", two=2)
                        nc.scalar.activation(
                            ptv[:, :, qoff:QB],
                            spsv[:, :, qoff:QB],
                            AF.Exp,
                            scale=float(1.0 / np.sqrt(DH)),
                        )
                        if i >= 0:
                            # causal mask applied post-exp as a 0/1 multiply:
                            # keeps the DVE op off the scores->exp chain (it
                            # only gates PV, which lags two iterations)
                            nc.vector.tensor_mul(
                                ptv[:, :, qoff : qoff + 128],
                                ptv[:, :, qoff : qoff + 128],
                                trid[:].rearrange("p (two c) -> p two c", two=2),
                            )
                        emit_pv()
                        prev_pv = (aps, hA, hB, pt, qoff, t, t == 4 * j + 3)
                        if t % 2 == 1 and bfill:
                            bfill.pop(0)[1]()
                            emit_norm()
                        elif t == 3:
                            emit_norm()
                    for _, f in bfill:
                        f()
                    emit_pv()
                    pending_norms.append((aps, hA, hB, j))
                while pending_norms:
                    emit_norm()
            if hw_loop and repeat > 1:
                assert repeat % 2 == 1, "2-body unroll expects odd repeat"
                with tc.For_i(0, repeat // 2, 1):
                    emit_body()
                    emit_body()
                emit_body()
            else:
                for _rep in range(repeat):
                    emit_body()

    _dedup_ldweights(nc)
    _split_multi_waits(nc)
    return nc


def _get_runner():
    if "nc" not in _CACHE:
        _CACHE["nc"] = build_module()
    return _CACHE["nc"]


def _make_in_maps(x, Wq, bq, Wk, bk, Wv, bv):
    import ml_dtypes

    bf16 = ml_dtypes.bfloat16
    x = np.asarray(x, dtype=np.float32)
    Wq = np.asarray(Wq, dtype=bf16)
    Wk = np.asarray(Wk, dtype=bf16)
    Wv = np.asarray(Wv, dtype=bf16)
    bq = np.asarray(bq, dtype=np.float32)
    bk = np.asarray(bk, dtype=np.float32)
    bv = np.asarray(bv, dtype=np.float32)

    kp = np.arange(128)[:, None]
    qf = np.arange(128)[None, :]
    tri = np.where(kp <= qf, 1.0, 0.0).astype(bf16)
    trid = np.concatenate([tri, tri], axis=1)
    ones = np.ones((128, 4), np.float32)

    xTs = [np.ascontiguousarray(x[b].T.astype(bf16)) for b in range(B)]
    in_maps = []
    for c in range(N_CORES):
        b = c // 4
        g = c % 4
        sl = slice(DCORE * g, DCORE * (g + 1))
        in_maps.append(
            {
                "xT": xTs[b],
                "wq": np.ascontiguousarray(Wq[:, sl]),
                "wk": np.ascontiguousarray(Wk[:, sl]),
                "wv": np.ascontiguousarray(Wv[:, sl]),
                "bq": np.ascontiguousarray(bq[sl]),
                "bk": np.ascontiguousarray(bk[sl]),
                "bv": np.ascontiguousarray(bv[sl]),
                "tri": trid,
                "ones": ones,
            }
        )
    return in_maps


def kernel(x, Wq, bq, Wk, bk, Wv, bv):
    from concourse.bass_utils import run_bass_kernel_spmd

    nc = _get_runner()
    in_maps = _make_in_maps(x, Wq, bq, Wk, bk, Wv, bv)
    res = run_bass_kernel_spmd(nc, in_maps, list(range(N_CORES)))
    out = np.empty((B, S, D), dtype=np.float32)
    for c in range(N_CORES):
        b = c // 4
        g = c % 4
        out[b, :, DCORE * g : DCORE * (g + 1)] = res.results[c]["outT"].T
    return out


# revision 41
# speedup vs baseline: 1.0752x; 1.0351x over previous
"""Multi-head causal attention (B=2, S=2048, D=1024, H=16) on 8 TRN2 NeuronCores.

Sharding: tensor-parallel over heads x data-parallel over batch.
Core c handles batch b = c // 4 and head group g = c % 4 (heads 4g..4g+3),
i.e. a [2048, 256] slice of the output.

Per-core kernel, bf16 data path (PSUM accumulation stays fp32):
  - x, W, Q^T, K^T, V', probabilities in bf16: matmuls run 1 cycle/row at
    any width and measure faster than fp32r on hardware even with the
    explicit ldweights; DMA bytes halve; DVE copies hit the 2x 16-bit mode.
  - Q^T/K^T projections produce d-major [256, 2048] activations directly
    (lhsT = W column slice, rhs = host-pretransposed x^T); a matmul output
    may not cross a PSUM bank, so each k contributes two 512-wide matmuls
    with the weight tile stationary across both.
  - V' is k-major [128, 4*65] with a ones column per head (the PV matmul
    then also yields softmax denominators); ones columns via gpsimd memset
    on the otherwise idle Pool engine, V bias via a K=1 ones x bias-row
    f32r matmul into the same PSUM accumulation.
  - Scores are computed transposed (S^T = K @ Q^T): softmax needs no
    transposes. A head pair shares one 128-row Q^T/K^T tile; the two K=64
    score matmuls go to distinct PE row groups (tile_position (0,0)/(64,0))
    and the two halves of one [128,1024] PSUM tile, sharing a single
    strided exp on ScalarE. Causal masking is a 0/1 bf16 multiply on the
    probabilities AFTER the exp, keeping VectorE latency off the
    scores->exp chain (it only gates PV, which lags two iterations).
  - Redundant InstLdweights (consecutive matmuls sharing a stationary
    tile) are stripped post-hoc; they are unmodeled in CoreSim but cost
    real cycles on hardware.
  - Normalization: 1/s = exp(-ln(s)) on ScalarE, emitted right after a
    filler chain whose PSUM-slot hold pauses the exp stream anyway
    (custom-DVE approx reciprocal fails this walrus build, plain DVE
    reciprocal runs ~7 cycles/element on hardware, gpsimd divide is not
    a legal Pool opcode); bf16 broadcast across partitions by a K=1 ones
    matmul, one VectorE multiply.
  - Schedule: software-pipelined emission. PV lags two t-iterations behind
    its exp so upcoming scores never sit behind a stalled PV in the
    in-order PE queue; V'/projection chains are woven between t-iterations
    as PE filler, with the tail blocks carrying the NEXT loop iteration's
    lead-in chains so the steady-state body starts attention immediately.

Ruled out: fp8e4 DoubleRow PV (would halve PV stream cycles and
instruction count). Compile probes show fp8e4 exp output, f32->fp8
tensor_copy, fp8 memset and all-fp8 tensor_mul all work, but the
DoubleRow/DoubleRowSwInterleave matmul Ldweights fails walrus codegen's
ISA check in this build for every M tried (64/65/66) - the perf mode is
unsupported here, so the PV matmul cannot go below 1 cycle/row.
"""

import os
import sys

import numpy as np

for _p in ("/opt/trn_rl_repo", "/root/.axon_site/_ro/trn_rl_repo"):
    if os.path.isdir(_p) and _p not in sys.path:
        sys.path.insert(0, _p)

B, S, D, H = 2, 2048, 1024, 16
N_CORES = 8
HEADS_PER_CORE = 4
DH = D // H  # 64
DCORE = HEADS_PER_CORE * DH  # 256
KT = D // 128  # 8 contraction tiles for the projections
ST = S // 128  # 16 sequence tiles
QB = 512  # q block width
NEG = -1.0e30

_CACHE = {}


def _split_multi_waits(nc, max_waits=1):
    """This walrus build rejects instructions carrying more than one
    semaphore wait; hoist extras onto preceding NoOps on the same engine."""
    import bass_rust as _br

    n = 0
    for fn in nc.m.functions:
        for bb in fn.blocks:
            insts = list(bb.instructions)
            new = []
            changed = False
            for inst in insts:
                si = getattr(inst, "sync_info", None)
                ow = list(si.on_wait) if si is not None else []
                if len(ow) > max_waits:
                    changed = True
                    for w in ow[:-max_waits]:
                        n += 1
                        new.append(
                            _br.InstNoOp(
                                name=f"I-ws{n}",
                                engine=inst.engine,
                                ins=[],
                                outs=[],
                                sync_info=_br.SyncInfo(on_wait=[w], on_update=[]),
                            )
                        )
                    si.on_wait = ow[-max_waits:]
                    inst.sync_info = si
                new.append(inst)
            if changed:
                bb.instructions = new


def _dedup_ldweights(nc):
    """Drop an InstLdweights whose weights operand is identical to the one
    already loaded (qk_chain emits ldw/mm pairs sharing a stationary tile).
    Non-self-loading matmuls leave the PE weight registers intact; fp32/f32r
    matmuls self-load, so they invalidate the tracked key."""
    import concourse.mybir as mybir

    for fn in nc.m.functions:
        for bb in fn.blocks:
            last_key = None
            new = []
            for inst in bb.instructions:
                tn = type(inst).__name__
                if str(getattr(inst, "engine", None)) == "EngineType.PE":
                    if tn == "InstLdweights":
                        try:
                            w = inst.ins[0]
                            key = (
                                w.memref,
                                w.offset,
                                str(w.ap),
                                str(getattr(inst, "tile_position", None)),
                                str(getattr(inst, "perf_mode", None)),
                            )
                        except Exception:
                            key = None
                        si = getattr(inst, "sync_info", None)
                        waits = list(si.on_wait) if si else []
                        ups = list(si.on_update) if si else []
                        if key is not None and key == last_key and not waits and not ups:
                            continue
                        last_key = key
                    elif tn == "InstMatmult":
                        try:
                            if inst.ins[1].dtype in (
                                mybir.dt.float32,
                                mybir.dt.float32r,
                            ):
                                last_key = None
                        except Exception:
                            last_key = None
                    elif tn in ("InstNoOp", "InstEventSemaphore", "InstRegisterMove"):
                        pass
                    else:
                        last_key = None
                new.append(inst)
            bb.instructions = new


def build_module(repeat=1, hw_loop=False):
    import contextlib

    import concourse.bass as bass
    import concourse.mybir as mybir
    from concourse.tile import TileContext

    F32 = mybir.dt.float32
    F32R = mybir.dt.float32r
    BF16 = mybir.dt.bfloat16
    AF = mybir.ActivationFunctionType

    nc = bass.Bass("TRN2", target_bir_lowering=False, debug=False, num_devices=N_CORES)

    xT_in = nc.declare_dram_parameter("xT", [D, S], BF16, isOutput=False)
    wq_in = nc.declare_dram_parameter("wq", [D, DCORE], BF16, isOutput=False)
    wk_in = nc.declare_dram_parameter("wk", [D, DCORE], BF16, isOutput=False)
    wv_in = nc.declare_dram_parameter("wv", [D, DCORE], BF16, isOutput=False)
    bq_in = nc.declare_dram_parameter("bq", [DCORE], F32, isOutput=False)
    bk_in = nc.declare_dram_parameter("bk", [DCORE], F32, isOutput=False)
    bv_in = nc.declare_dram_parameter("bv", [DCORE], F32, isOutput=False)
    tri_in = nc.declare_dram_parameter("tri", [128, 256], BF16, isOutput=False)
    ones_in = nc.declare_dram_parameter("ones", [128, 4], F32, isOutput=False)
    outT = nc.declare_dram_parameter("outT", [DCORE, S], F32, isOutput=True)

    with TileContext(nc) as tc:
        with (
            tc.tile_pool(name="persist", bufs=1) as pp,
            tc.tile_pool(name="work", bufs=4) as wp,
            tc.tile_pool(name="outp", bufs=3) as op,
            tc.tile_pool(name="mm_ps", bufs=2, space="PSUM") as mm_ps,
            tc.tile_pool(name="att_ps", bufs=2, space="PSUM") as att_ps,
        ):
            # ---- constant / persistent tiles -------------------------------
            trid = pp.tile([128, 256], BF16, tag="trid")
            nc.sync.dma_start(trid[:], tri_in[:])
            onesr = pp.tile([1, 128], F32R, tag="onesr")  # K=1 matmul lhsT
            nc.sync.dma_start(
                onesr[:], ones_in[:, 0:1].rearrange("p a -> a p").bitcast(F32R)
            )
            bvrow = pp.tile([1, DCORE], F32R, tag="bvrow")
            nc.sync.dma_start(
                bvrow[:], bv_in[:].rearrange("(a b) -> a b", a=1).bitcast(F32R)
            )
            bqc = pp.tile([128, 2], F32, tag="bqc")
            nc.sync.dma_start(bqc[:], bq_in[:].rearrange("(m p) -> p m", p=128))
            bkc = pp.tile([128, 2], F32, tag="bkc")
            nc.sync.dma_start(bkc[:], bk_in[:].rearrange("(m p) -> p m", p=128))
            onesb = pp.tile([1, 64], BF16, tag="onesb")  # bf16 bcast lhsT
            nc.vector.tensor_copy(onesb[:], onesr[:, 0:64].bitcast(F32))

            wq = []
            wk = []
            wv = []
            for k in range(KT):
                for name, lst, src in (("wq", wq, wq_in), ("wk", wk, wk_in), ("wv", wv, wv_in)):
                    t = pp.tile([128, DCORE], BF16, tag=f"{name}{k}")
                    nc.sync.dma_start(t[:], src[128 * k : 128 * (k + 1), :])
                    lst.append(t)
            # ---- warmup during the x DMA window: dummy matmuls ramp the PE
            # HAM clock gate to 2.4 GHz; one exp pulls the activation table
            # load off the critical path ------------------------------------
            warm_ps = mm_ps.tile([128, 2 * QB], F32, tag="mm", name="warm_ps")
            for _w in range(42):
                nc.tensor.matmul(
                    warm_ps[:, 0:DCORE], onesr[:], bvrow[:], start=True, stop=True
                )
            warm_o = wp.tile([1, 128], F32, tag="warm", name="warm_o")
            nc.scalar.activation(warm_o[:], onesr[:].bitcast(F32), AF.Exp)

            # x^T tiles, loaded in [128, QB] slices n-major so the first
            # projection blocks can start after ~1/4 of x has landed
            xt = [pp.tile([128, S], BF16, tag=f"xt{k}", name=f"xt{k}") for k in range(KT)]
            for n in range(S // QB):
                for k in range(KT):
                    nc.sync.dma_start(
                        xt[k][:, QB * n : QB * (n + 1)],
                        xT_in[128 * k : 128 * (k + 1), QB * n : QB * (n + 1)],
                    )

            qT = [pp.tile([128, S], BF16, tag=f"qT{m}", name=f"qT{m}") for m in range(2)]
            kTt = [pp.tile([128, S], BF16, tag=f"kT{m}", name=f"kT{m}") for m in range(2)]
            vp = [pp.tile([128, 4 * 65], BF16, tag=f"vp{s}", name=f"vp{s}") for s in range(ST)]

            def v_chain(s):
                # one V' sequence tile: k-major [128, 4*65] with a ones
                # column per head (PV then also yields softmax denoms)
                dst = vp[s]
                nc.gpsimd.memset(
                    dst[:].rearrange("p (h c) -> p h c", c=65)[:, :, 64:65], 1.0
                )
                ps = mm_ps.tile([128, DCORE], F32, tag="mm", name=f"vps{s}")
                for k in range(KT):
                    nc.tensor.matmul(
                        ps[:],
                        xt[k][:, 128 * s : 128 * (s + 1)],
                        wv[k][:],
                        start=(k == 0),
                        stop=False,
                    )
                nc.tensor.matmul(ps[:], onesr[:], bvrow[:], start=False, stop=True)
                nc.vector.tensor_copy(
                    dst[:].rearrange("p (h c) -> p h c", c=65)[:, :, 0:64],
                    ps[:].rearrange("p (h c) -> p h c", c=64),
                )

            def qk_chain(lst, w, bias, m, half):
                # one [128,1024]-wide accumulation chain of a Q/K proj;
                # matmul outputs may not cross a PSUM bank, so each k
                # contributes two 512-wide matmuls (weights stationary)
                acc = mm_ps.tile([128, 2 * QB], F32, tag="mm", name="acc")
                for k in range(KT):
                    for h2 in range(2):
                        nc.tensor.matmul(
                            acc[:, QB * h2 : QB * (h2 + 1)],
                            w[k][:, 128 * m : 128 * (m + 1)],
                            xt[k][
                                :,
                                2 * QB * half + QB * h2 : 2 * QB * half
                                + QB * (h2 + 1),
                            ],
                            start=(k == 0),
                            stop=(k == KT - 1),
                        )
                nc.vector.tensor_scalar_add(
                    lst[m][:, 2 * QB * half : 2 * QB * (half + 1)],
                    acc[:],
                    bias[:, m : m + 1],
                )

            # ---- lead-in (once): V' s0-3 + Q/K m0 half0 unlock the first two
            # attention blocks; in the repeat loop these chains for the NEXT
            # iteration are emitted as fillers in the tail blocks, so the
            # steady-state body starts its attention immediately ------------
            v_chain(0)
            v_chain(1)
            v_chain(2)
            v_chain(3)
            qk_chain(qT, wq, bqc, 0, 0)
            qk_chain(kTt, wk, bkc, 0, 0)

            def emit_body():
                # filler chains woven between t-iterations (one per two t's),
                # ordered so each completes before its consumer block; the
                # tail blocks carry the next iteration's lead-in chains
                per_block_fillers = {
                    0: [
                        (False, lambda: v_chain(4)),
                        (True, lambda: qk_chain(qT, wq, bqc, 1, 0)),
                    ],
                    1: [
                        (False, lambda: v_chain(5)),
                        (True, lambda: qk_chain(kTt, wk, bkc, 1, 0)),
                        (False, lambda: v_chain(6)),
                        (False, lambda: v_chain(7)),
                    ],
                    2: [
                        (True, lambda: qk_chain(qT, wq, bqc, 0, 1)),
                        (True, lambda: qk_chain(kTt, wk, bkc, 0, 1)),
                    ],
                    3: [
                        (False, lambda: v_chain(8)),
                        (False, lambda: v_chain(9)),
                        (False, lambda: v_chain(10)),
                        (False, lambda: v_chain(11)),
                    ],
                    4: [
                        (True, lambda: qk_chain(qT, wq, bqc, 1, 1)),
                        (True, lambda: qk_chain(kTt, wk, bkc, 1, 1)),
                        (False, lambda: v_chain(12)),
                        (False, lambda: v_chain(13)),
                        (False, lambda: v_chain(14)),
                        (False, lambda: v_chain(15)),
                    ],
                    6: [
                        (True, lambda: qk_chain(qT, wq, bqc, 0, 0)),
                        (True, lambda: qk_chain(kTt, wk, bkc, 0, 0)),
                    ],
                    7: [
                        (False, lambda: v_chain(0)),
                        (False, lambda: v_chain(1)),
                        (True, lambda: v_chain(2)),
                        (False, lambda: v_chain(3)),
                    ],
                }

                # ---- attention, software-pipelined emission ---------------
                blocks = [(0, 0), (0, 1), (1, 0), (1, 1), (0, 2), (0, 3), (1, 2), (1, 3)]
                prev_pv = None  # (aps, hA, hB, pt, qoff, t, last)
                pending_norms = []  # FIFO of (aps, hA, hB, j)

                def emit_pv():
                    nonlocal prev_pv
                    if prev_pv is None:
                        return
                    aps, hA, hB, pt, qoff, t, last = prev_pv
                    for h, off in ((hA, 0), (hB, QB)):
                        nc.tensor.matmul(
                            aps[0:65, off + qoff : off + QB],
                            vp[t][:, 65 * h : 65 * h + 65],
                            pt[:, off + qoff : off + QB],
                            start=(t == 0),
                            stop=last,
                        )
                    prev_pv = None

                def emit_norm():
                    if not pending_norms:
                        return
                    aps, hA, hB, j = pending_norms.pop(0)
                    # 1/s = exp(-ln(s)) on ScalarE, emitted right after a
                    # filler chain whose PSUM-slot hold pauses the exp
                    # stream anyway. (Alternatives that fail here: custom-DVE
                    # approx reciprocal -> "ISA wrong length" in this walrus
                    # build; plain DVE reciprocal -> ~7 cycles/element on
                    # hardware; gpsimd divide -> not a legal Pool opcode.)
                    lns = wp.tile([1, 2 * QB], F32, tag="lns", name="lns")
                    nc.scalar.activation(lns[:], aps[64:65, :], AF.Ln)
                    rrow = wp.tile([1, 2 * QB], BF16, tag="rrow", name="rrow")
                    nc.scalar.activation(rrow[:], lns[:], AF.Exp, scale=-1.0)
                    rbp = mm_ps.tile([128, 2 * QB], F32, tag="mm", name="rbp")
                    for h2 in range(2):
                        nc.tensor.matmul(
                            rbp[0:64, QB * h2 : QB * (h2 + 1)],
                            onesb[:],
                            rrow[:, QB * h2 : QB * (h2 + 1)],
                            start=True,
                            stop=True,
                        )
                    rb = wp.tile([64, 2 * QB], F32, tag="rb", name="rb")
                    nc.vector.tensor_copy(rb[:], rbp[0:64, :])
                    att = op.tile([64, 2 * QB], F32, tag="att_out", name="att")
                    nc.vector.tensor_mul(att[:], aps[0:64, :], rb[:])
                    for h, off in ((hA, 0), (hB, QB)):
                        nc.sync.dma_start(
                            outT[64 * h : 64 * (h + 1), QB * j : QB * (j + 1)],
                            att[:, off : off + QB],
                        )

                for bi, (hp, j) in enumerate(blocks):
                    hA, hB = 2 * hp, 2 * hp + 1
                    qTm, kTm = qT[hp], kTt[hp]
                    bfill = list(per_block_fillers.get(bi, ()))
                    aps = att_ps.tile([128, 2 * QB], F32, tag="att", name=f"aps{hp}_{j}")
                    for t in range(4 * j + 4):
                        i = t - 4 * j  # >= 0 only on diagonal-region tiles
                        qoff = 128 * max(i, 0)
                        qwin = slice(QB * j + qoff, QB * (j + 1))
                        ktile = slice(128 * t, 128 * (t + 1))
                        sps = mm_ps.tile([128, 2 * QB], F32, tag="mm", name="sps")
                        nc.tensor.matmul(
                            sps[:, qoff:QB],
                            kTm[0:64, ktile],
                            qTm[0:64, qwin],
                            start=True,
                            stop=True,
                            tile_position=(0, 0),
                        )
                        nc.tensor.matmul(
                            sps[:, QB + qoff : 2 * QB],
                            kTm[64:128, ktile],
                            qTm[64:128, qwin],
                            start=True,
                            stop=True,
                            tile_position=(64, 0),
                        )
                        spsv = sps[:].rearrange("p (two c) -> p two c", two=2)
                        pt = wp.tile([128, 2 * QB], BF16, tag="pt")
                        ptv = pt[:].rearrange("p (two c) -> p two c", two=2)
                        nc.scalar.activation(
                            ptv[:, :, qoff:QB],
                            spsv[:, :, qoff:QB],
                            AF.Exp,
                            scale=float(1.0 / np.sqrt(DH)),
                        )
                        if i >= 0:
                            # causal mask applied post-exp as a 0/1 multiply:
                            # keeps the DVE op off the scores->exp chain (it
                            # only gates PV, which lags two iterations)
                            nc.vector.tensor_mul(
                                ptv[:, :, qoff : qoff + 128],
                                ptv[:, :, qoff : qoff + 128],
                                trid[:].rearrange("p (two c) -> p two c", two=2),
                            )
                        emit_pv()
                        prev_pv = (aps, hA, hB, pt, qoff, t, t == 4 * j + 3)
                        if t % 2 == 1 and bfill:
                            bfill.pop(0)[1]()
                            emit_norm()
                        elif t == 3:
                            emit_norm()
                    for _, f in bfill:
                        f()
                    emit_pv()
                    pending_norms.append((aps, hA, hB, j))
                while pending_norms:
                    emit_norm()
            if hw_loop and repeat > 1:
                UNROLL = 4
                assert repeat > UNROLL and repeat % UNROLL == 1, "unroll wants repeat = 4k+1"
                with tc.For_i(0, repeat // UNROLL, 1):
                    for _u in range(UNROLL):
                        emit_body()
                emit_body()
            else:
                for _rep in range(repeat):
                    emit_body()

    _dedup_ldweights(nc)
    _split_multi_waits(nc)
    return nc


def _get_runner():
    if "nc" not in _CACHE:
        _CACHE["nc"] = build_module()
    return _CACHE["nc"]


def _make_in_maps(x, Wq, bq, Wk, bk, Wv, bv):
    import ml_dtypes

    bf16 = ml_dtypes.bfloat16
    x = np.asarray(x, dtype=np.float32)
    Wq = np.asarray(Wq, dtype=bf16)
    Wk = np.asarray(Wk, dtype=bf16)
    Wv = np.asarray(Wv, dtype=bf16)
    bq = np.asarray(bq, dtype=np.float32)
    bk = np.asarray(bk, dtype=np.float32)
    bv = np.asarray(bv, dtype=np.float32)

    kp = np.arange(128)[:, None]
    qf = np.arange(128)[None, :]
    tri = np.where(kp <= qf, 1.0, 0.0).astype(bf16)
    trid = np.concatenate([tri, tri], axis=1)
    ones = np.ones((128, 4), np.float32)

    xTs = [np.ascontiguousarray(x[b].T.astype(bf16)) for b in range(B)]
    in_maps = []
    for c in range(N_CORES):
        b = c // 4
        g = c % 4
        sl = slice(DCORE * g, DCORE * (g + 1))
        in_maps.append(
            {
                "xT": xTs[b],
                "wq": np.ascontiguousarray(Wq[:, sl]),
                "wk": np.ascontiguousarray(Wk[:, sl]),
                "wv": np.ascontiguousarray(Wv[:, sl]),
                "bq": np.ascontiguousarray(bq[sl]),
                "bk": np.ascontiguousarray(bk[sl]),
                "bv": np.ascontiguousarray(bv[sl]),
                "tri": trid,
                "ones": ones,
            }
        )
    return in_maps


def kernel(x, Wq, bq, Wk, bk, Wv, bv):
    from concourse.bass_utils import run_bass_kernel_spmd

    nc = _get_runner()
    in_maps = _make_in_maps(x, Wq, bq, Wk, bk, Wv, bv)
    res = run_bass_kernel_spmd(nc, in_maps, list(range(N_CORES)))
    out = np.empty((B, S, D), dtype=np.float32)
    for c in range(N_CORES):
        b = c // 4
        g = c % 4
        out[b, :, DCORE * g : DCORE * (g + 1)] = res.results[c]["outT"].T
    return out
